# revision 2
# baseline (speedup 1.0000x reference)
"""Trainium2 Bass kernel for the permutation-invariant L1 loss (PIT loss).

Math: pairwise[b,j,k] = mean_{T,F} |pred[b,j] - tgt[b,k]|, then a 6-perm
min/argmin per batch.  The device only computes S_max[b,j,k] =
sum max(pred[b,j], tgt[b,k]) via one fused DVE scalar_tensor_tensor
(op0=bypass, op1=max, accum_out=running fp32 sum) per pair; the identity
sum|a-b| = 2*sum max(a,b) - sum a - sum b recovers the L1 sums with
per-slab sums computed on the host (cheap, and permutation-invariant so
they don't affect the argmin).  Inputs are cast to fp16 on the host:
halves HBM traffic and doubles DVE throughput; max() is exact in fp16 and
the reduction accumulates in fp32, so end-to-end error is ~1e-6 rel.

Toolchain notes: this walrus build rejects instructions carrying >1 sync
wait ("Too many sync wait commands"), so (a) the Tile kernel-tail drain is
re-emitted as one single-wait drain per outstanding semaphore, (b) each
batch is loaded by ONE interleaved DMA (single sem for the first consumer),
and (c) every STT writes through a stride-0 broadcast AP onto a unique tiny
tile so no scratch-reuse WAR wait is ever paired with the DMA wait.
"""

import sys

sys.path.insert(0, "/opt/trn_rl_repo")

import numpy as np

B, S, T, F = 32, 3, 1000, 257
N_CORES = 8
BPC = B // N_CORES  # batches per core
NTF = T * F  # 257000
P, FD = 125, 2056  # 125 * 2056 == T * F, so a slab is one [P, FD] tile
NPAIR = S * S
PERMS = np.array(
    [[0, 1, 2], [0, 2, 1], [1, 0, 2], [1, 2, 0], [2, 0, 1], [2, 1, 0]],
    dtype=np.int32,
)

_CACHE = {}
LAST_RESULTS = None


def _patch_tail_drain():
    """Split the Tile kernel-tail drain (one wait per live semaphore lane)
    into chained single-wait drains; this walrus build caps sync waits per
    instruction."""
    from concourse.tile import TileContext
    from concourse.vector_clock import ScopedClock, VectorClock

    if getattr(TileContext, "_tail_drain_patched", False):
        return

    def _drain_and_barrier(self, tick_clock, wait_clock):
        gc = list(tick_clock.global_clock)
        for i, v in enumerate(gc):
            if v:
                single = [0] * len(gc)
                single[i] = v
                inst = self.nc.sync.drain()
                wait_clock.add_sem_waits(
                    inst.ins, ScopedClock({None: VectorClock(single)})
                )
        self.nc.all_engine_barrier()
        assert self.sems is not None
        popped = self.nc._tile_sem_poison_stack.pop()
        assert popped is self._sem_poison
        self.nc.clear_and_free_semaphores(list(self.sems.allocated().values()))
        self.nc.all_engine_barrier()

    TileContext._drain_and_barrier = _drain_and_barrier
    TileContext._tail_drain_patched = True


def _build_bass():
    import concourse.mybir as mybir
    from concourse.bass import Bass
    from concourse.tile import TileContext

    _patch_tail_drain()
    nc = Bass()
    x = nc.dram_tensor(
        "x", [BPC * 2 * S * P, FD], mybir.dt.float16, kind="ExternalInput"
    )
    o = nc.dram_tensor("o", [P, BPC * NPAIR], mybir.dt.float32, kind="ExternalOutput")

    with TileContext(nc) as tc:
        with (
            tc.tile_pool(name="io", bufs=4) as iop,
            tc.tile_pool(name="scp", bufs=40) as scp,
            tc.tile_pool(name="stp", bufs=1) as stp,
        ):
            st = stp.tile([P, BPC * NPAIR], mybir.dt.float32, name="st")
            for b in range(BPC):
                tx = iop.tile(
                    [P, 2 * S * FD], mybir.dt.float16, tag="tx", name=f"tx{b}"
                )
                r0 = b * 2 * S * P
                nc.sync.dma_start(
                    out=tx[:].rearrange("p (s f) -> p s f", s=2 * S),
                    in_=x[r0 : r0 + 2 * S * P, :].rearrange(
                        "(s p) f -> p s f", s=2 * S
                    ),
                )
                for j in range(S):
                    for k in range(S):
                        sc = scp.tile([P, 1], mybir.dt.float16, name=f"sc{b}_{j}{k}")
                        col = b * NPAIR + j * S + k
                        nc.vector.scalar_tensor_tensor(
                            out=sc[:].broadcast_to((P, FD)),
                            in0=tx[:, j * FD : (j + 1) * FD],
                            scalar=0.0,
                            in1=tx[:, (S + k) * FD : (S + k + 1) * FD],
                            op0=mybir.AluOpType.bypass,
                            op1=mybir.AluOpType.max,
                            accum_out=st[:, col : col + 1],
                        )
            nc.sync.dma_start(out=o[:, :], in_=st[:])
    return nc


def kernel(predicted: np.ndarray, padded_srcs_feats: np.ndarray):
    global LAST_RESULTS
    from concourse.bass_utils import run_bass_kernel_spmd

    if "nc" not in _CACHE:
        _CACHE["nc"] = _build_bass()
    nc = _CACHE["nc"]

    pred16 = np.ascontiguousarray(predicted, dtype=np.float32).astype(np.float16)
    tgt16 = np.ascontiguousarray(padded_srcs_feats, dtype=np.float32).astype(
        np.float16
    )

    # Per-slab sums of the fp16-rounded values (fp64 accumulate) so the
    # max-identity is exact w.r.t. what the device saw.
    psum = pred16.reshape(B, S, NTF).astype(np.float64).sum(axis=-1)  # [B, S]
    tsum = tgt16.reshape(B, S, NTF).astype(np.float64).sum(axis=-1)  # [B, S]

    # Interleave per batch: [pred slabs (3), tgt slabs (3)], each slab [P, FD].
    x = np.stack(
        [pred16.reshape(B, S * P, FD), tgt16.reshape(B, S * P, FD)], axis=1
    )  # [B, 2, S*P, FD]
    x = np.ascontiguousarray(x).reshape(N_CORES, BPC * 2 * S * P, FD)
    in_maps = [{"x": np.ascontiguousarray(x[c])} for c in range(N_CORES)]

    LAST_RESULTS = run_bass_kernel_spmd(nc, in_maps, core_ids=list(range(N_CORES)))
    results = LAST_RESULTS.results

    # [B, S, S]: S_max[b, j, k] = sum max(pred[b,j], tgt[b,k])
    smax = np.concatenate(
        [r["o"].astype(np.float64).sum(axis=0).reshape(BPC, S, S) for r in results],
        axis=0,
    )
    sum_abs = 2.0 * smax - psum[:, :, None] - tsum[:, None, :]
    pairwise = sum_abs / NTF  # [B, S, S]

    # loss_set[b, p] = mean_s pairwise[b, PERMS[p, s], s]
    loss_set = pairwise[:, PERMS, np.arange(S)].mean(axis=-1)  # [B, 6]
    min_idx = np.argmin(loss_set, axis=1)
    min_loss = loss_set[np.arange(B), min_idx]
    best_perms = PERMS[min_idx].astype(np.int32)
    return np.float32(min_loss.mean()), best_perms
